# revision 8
# baseline (speedup 1.0000x reference)
"""PodNet classifier head (retrieval kNN with per-class softmax pooling) on 8 trn2 cores.

Math (equivalent to the reference, validated in fp64/fp32):
    a    = 2 * x / ||x||              (factor 2 folded into the operand)
    thn  = theta_col / ||theta_col||  (columns indexed class-major: r = c*10 + j)
    s2   = a @ thn                    (= 2 * cosine similarity, in [-2, 2])
    e    = exp(s2)
    p    = (s2 - 2) * e               (the e^2 cancels in the ratio)
    out[b,c] = sum_j p / sum_j e

Sharding: batch 8192 split 8 ways (1024 rows per core); theta replicated.
Device layout: batch rows on partitions, class-major r on the free dim, so the
per-class softmax reductions are strided free-dim group reduces on DVE.
"""

import numpy as np
import orjson

import concourse.bass as bass
import concourse.mybir as mybir
import concourse.tile as tile
from concourse.bass_utils import run_bass_kernel_spmd
from concourse.masks import make_identity

F32 = mybir.dt.float32
BF16 = mybir.dt.bfloat16
AF = mybir.ActivationFunctionType
ALU = mybir.AluOpType

BATCH, D, K, C = 8192, 64, 10, 1000
R = C * K                # 10000
NCORES = 8
BC = BATCH // NCORES     # 1024 rows per core
P = 128
NB = BC // P             # 8 batch tiles per core
CH = 2000                # free-dim elems per main chunk (200 classes)
NCH = R // CH            # 5 chunks
CCH = CH // K            # 200 classes per chunk
NMM = 4                  # matmuls per chunk
MMN = CH // NMM          # 500 columns per matmul
TP = 125                 # theta-prep tile partitions
NT = R // TP             # 80 theta-prep tiles


# ---------------------------------------------------------------------------
# Workaround for this walrus build's 1-wait-per-instruction sync limit: for any
# instruction carrying N>1 sem waits, hoist N-1 waits onto preceding NoOps on
# the same engine (the engine's sequencer blocks on each in order, so the
# combined-AND semantics are preserved; updates stay on the real instruction).
def _fix_block(instructions: list) -> list:
    out = []
    for inst in instructions:
        sync = inst.get("sync_info") or {}
        waits = sync.get("on_wait") or []
        if len(waits) > 1:
            for i, w in enumerate(waits[:-1]):
                out.append(
                    {
                        "debug": inst.get("debug", 0),
                        "engine": inst["engine"],
                        "ins": [],
                        "name": f"{inst['name']}w{i}",
                        "opcode": "NoOp",
                        "outs": [],
                        "sync_info": {"on_wait": [w]},
                    }
                )
            inst = dict(inst)
            inst["sync_info"] = {
                **{k: v for k, v in sync.items() if k != "on_wait"},
                "on_wait": [waits[-1]],
            }
        out.append(inst)
    return out


def _walk_fix(obj):
    if isinstance(obj, dict):
        if isinstance(obj.get("instructions"), list):
            obj["instructions"] = _fix_block(obj["instructions"])
        for v in obj.values():
            _walk_fix(v)
    elif isinstance(obj, list):
        for v in obj:
            _walk_fix(v)


def _patch_bass(nc):
    orig = nc.to_json_bytes

    def fixed(*a, **k):
        m = orjson.loads(orig(*a, **k))
        _walk_fix(m)
        return orjson.dumps(m)

    nc.to_json_bytes = fixed
    return nc
# ---------------------------------------------------------------------------


def build_bass() -> bass.Bass:
    nc = bass.Bass(trn_type="TRN2")
    x = nc.dram_tensor("x", [BC, D], F32, kind="ExternalInput")
    th_t = nc.dram_tensor("thT", [R, D], F32, kind="ExternalInput")
    out = nc.dram_tensor("out", [BC, C], F32, kind="ExternalOutput")

    with tile.TileContext(nc) as tc:
        with tc.tile_pool(name="persist", bufs=1) as persist:
            ident = persist.tile([P, P], BF16)
            make_identity(nc, ident[:])

            theta_n = persist.tile([D, R], BF16)   # normalized theta, class-major
            a_t = persist.tile([D, BC], BF16)      # 2 * normalized x, transposed

            # ---------------- prep phase ----------------
            with (
                tc.tile_pool(name="prep", bufs=1) as prep,
                tc.tile_pool(name="prepw", bufs=4) as prepw,
                tc.tile_pool(name="psum_prep", bufs=4, space="PSUM") as psum_prep,
            ):
                # x: [1024, 64] -> 8 tiles [128, 64] side by side
                x_all = prep.tile([P, NB * D], F32)
                nc.sync.dma_start(
                    out=x_all[:].rearrange("p (n d) -> p n d", d=D),
                    in_=x[:].rearrange("(n p) d -> p n d", p=P),
                )
                # thetaT: [10000, 64] -> 80 tiles [125, 64] side by side
                tht_all = prep.tile([TP, NT * D], F32)
                nc.sync.dma_start(
                    out=tht_all[:].rearrange("p (n d) -> p n d", d=D),
                    in_=th_t[:].rearrange("(n p) d -> p n d", p=TP),
                )

                # row norms^2: square then grouped reduce (shared scratch)
                sq = prep.tile([P, NT * D], F32)
                nc.scalar.activation(sq[:, : NB * D], x_all[:], AF.Square)
                n2x = prep.tile([P, NB], F32)
                nc.vector.tensor_reduce(
                    out=n2x[:],
                    in_=sq[:, : NB * D].rearrange("p (n d) -> p n d", d=D),
                    axis=mybir.AxisListType.X,
                    op=ALU.add,
                )
                # rnx = 2/||x||  (Sqrt(0.25*n2) = ||x||/2, then 1/.)
                nx = prep.tile([P, NB], F32)
                nc.scalar.activation(nx[:], n2x[:], AF.Sqrt, scale=0.25)
                rnx = prep.tile([P, NB], F32)
                nc.vector.reciprocal(rnx[:], nx[:])
                # normalize + transpose x tiles -> a_t [64, 1024]
                for i in range(NB):
                    a_bf = prepw.tile([P, D], BF16, tag="abf")
                    nc.vector.tensor_scalar_mul(
                        a_bf[:], x_all[:, i * D : (i + 1) * D], rnx[:, i : i + 1]
                    )
                    ps = psum_prep.tile([D, P], BF16, tag="pst")
                    nc.tensor.transpose(ps[:], a_bf[:], ident[:])
                    nc.vector.tensor_copy(a_t[:, i * P : (i + 1) * P], ps[:])

                nc.scalar.activation(sq[:TP, :], tht_all[:], AF.Square)
                n2t = prep.tile([TP, NT], F32)
                nc.vector.tensor_reduce(
                    out=n2t[:],
                    in_=sq[:TP, :].rearrange("p (n d) -> p n d", d=D),
                    axis=mybir.AxisListType.X,
                    op=ALU.add,
                )
                # rnt = 1/||theta_col||
                nt_ = prep.tile([TP, NT], F32)
                nc.scalar.activation(nt_[:], n2t[:], AF.Sqrt)
                rnt = prep.tile([TP, NT], F32)
                nc.vector.reciprocal(rnt[:], nt_[:])

                # normalize + transpose theta tiles -> theta_n [64, 10000]
                for t in range(NT):
                    th_bf = prepw.tile([TP, D], BF16, tag="thbf")
                    nc.vector.tensor_scalar_mul(
                        th_bf[:], tht_all[:, t * D : (t + 1) * D], rnt[:, t : t + 1]
                    )
                    ps = psum_prep.tile([D, TP], BF16, tag="pst")
                    nc.tensor.transpose(ps[:], th_bf[:], ident[:TP, :TP])
                    nc.vector.tensor_copy(theta_n[:, t * TP : (t + 1) * TP], ps[:])

            # ---------------- main phase ----------------
            with (
                tc.tile_pool(name="psum_main", bufs=2, space="PSUM") as psum_main,
                tc.tile_pool(name="ework", bufs=3) as ework,
                tc.tile_pool(name="gf", bufs=2) as gfpool,
                tc.tile_pool(name="outp", bufs=2) as outp,
            ):
                for i in range(NB):
                    g_t = gfpool.tile([P, C], F32, tag="g")
                    f_t = gfpool.tile([P, C], F32, tag="f")
                    for k in range(NCH):
                        # 4 bank-aligned matmuls of 500 cols into one 4-bank tile
                        ps = psum_main.tile([P, NMM * 512], F32, tag="ps")
                        for q in range(NMM):
                            nc.tensor.matmul(
                                ps[:, q * 512 : q * 512 + MMN],
                                lhsT=a_t[:, i * P : (i + 1) * P],
                                rhs=theta_n[:, k * CH + q * MMN : k * CH + (q + 1) * MMN],
                                start=True,
                                stop=True,
                            )
                        ps_v = ps[:].rearrange("p (q n) -> p q n", q=NMM)[:, :, :MMN]
                        e_t = ework.tile([P, CH], F32, tag="e")
                        nc.scalar.activation(e_t[:], ps_v, AF.Exp)
                        p_t = ework.tile([P, CH], F32, tag="p")
                        nc.vector.scalar_tensor_tensor(
                            out=p_t[:],
                            in0=ps_v,
                            scalar=-2.0,
                            in1=e_t[:],
                            op0=ALU.add,
                            op1=ALU.mult,
                        )
                        nc.vector.tensor_reduce(
                            out=g_t[:, k * CCH : (k + 1) * CCH],
                            in_=e_t[:].rearrange("p (c j) -> p c j", j=K),
                            axis=mybir.AxisListType.X,
                            op=ALU.add,
                        )
                        nc.vector.tensor_reduce(
                            out=f_t[:, k * CCH : (k + 1) * CCH],
                            in_=p_t[:].rearrange("p (c j) -> p c j", j=K),
                            axis=mybir.AxisListType.X,
                            op=ALU.add,
                        )
                    # out = f / g via rg = exp(-ln(g)) (cheap reciprocal; g > 0,
                    # f < 0 so the sign rides through the multiply)
                    lg = outp.tile([P, C], F32, tag="lg")
                    nc.scalar.activation(lg[:], g_t[:], AF.Ln)
                    rg = outp.tile([P, C], F32, tag="rg")
                    nc.scalar.activation(rg[:], lg[:], AF.Exp, scale=-1.0)
                    o_t = outp.tile([P, C], F32, tag="o")
                    nc.vector.tensor_tensor(o_t[:], f_t[:], rg[:], op=ALU.mult)
                    nc.sync.dma_start(out=out[i * P : (i + 1) * P, :], in_=o_t[:])
    _patch_bass(nc)
    return nc


_NC_CACHE: list = []
TRACE = False          # set True (e.g. from test.py) to capture an NTFF profile
LAST_RESULT: list = []  # BassKernelResults of the most recent run, for test.py


def kernel(x: np.ndarray, theta: np.ndarray) -> np.ndarray:
    assert x.shape == (BATCH, D) and theta.shape == (D, K, C)
    if not _NC_CACHE:
        _NC_CACHE.append(build_bass())
    nc = _NC_CACHE[0]

    # class-major flat theta, transposed: thT[c*K+j, d] = theta[d, j, c]
    th_cm_t = np.ascontiguousarray(
        theta.astype(np.float32).transpose(2, 1, 0).reshape(R, D)
    )
    in_maps = [
        {
            "x": np.ascontiguousarray(x[c * BC : (c + 1) * BC]).astype(np.float32),
            "thT": th_cm_t,
        }
        for c in range(NCORES)
    ]
    res = run_bass_kernel_spmd(
        nc, in_maps, core_ids=list(range(NCORES)), trace=TRACE
    )
    LAST_RESULT.clear()
    LAST_RESULT.append(res)
    return np.concatenate([r["out"] for r in res.results], axis=0)
